# revision 24
# baseline (speedup 1.0000x reference)
"""Multi-head attention Trainium2 kernel (B=4, S=2048, E=1024, H=16).

Sharding: 8 cores = 4 batch groups x 2-way head tensor-parallel.
Core c handles batch b=c//2 and heads [g*8, g*8+8) with g=c%2.
Partial output projections are pair-summed with a chunked ReduceScatter
(5 chunks, fired as soon as each 512/256-row block's projection lands),
so the exposed collective tail is one 256-row chunk.

v2 layout highlights:
- Scores run as 64x64 quadrant-packed matmuls: a head PAIR shares one
  512-col moving stream (head A's d on partitions 0-63, head B's on
  64-127), roughly halving scores PE time vs the padded-K=128 form.
- QT/KT tiles hold a head pair per 128 partitions (d of head A on
  0-63, head B on 64-127) which is exactly the projection PSUM layout,
  so each projection block needs a single [128,512] copy.
- Q/K/V projections and the output projection are emitted as PE filler
  inside the attention kb-loops, so the ACT engine (exp, the
  second-busiest engine) starts ~20us into the kernel instead of
  after a serial projection phase.
- exp runs once per (pair, 512-q strip, key block) on a [128,1024]
  PSUM tile (head A cols 0-511, head B 512-1023); V carries a ones
  column so P@V also emits the softmax denominator row.
- All matmuls in bf16 (fp32 PSUM accumulate).
"""

import os
import sys

import numpy as np

for _p in ("/opt/trn_rl_repo", "/root/.axon_site/_ro/trn_rl_repo"):
    if os.path.isdir(_p) and _p not in sys.path:
        sys.path.append(_p)

import ml_dtypes  # noqa: E402
from concourse import bacc, mybir, tile  # noqa: E402
from concourse.bass_utils import run_bass_kernel_spmd  # noqa: E402

B, S, E, H, DH = 4, 2048, 1024, 16, 64
N_CORES = 8
TP = 2  # head-parallel factor within a batch
H_LOC = H // TP  # 8 heads per core
EI_LOC = H_LOC * DH  # 512 local rows of the concat dim
N_SB = S // 128  # 16 token blocks
N_EC = E // 128  # 8 contraction chunks
N_QB = S // 512  # 4 query strips of 512
N_KB = S // 128  # 16 key blocks
N_HP = H_LOC // 2  # 4 head pairs
# ReduceScatter chunks: (row0, nrows, fire_after_sb)
RS_CHUNKS = [(0, 512, 3), (512, 512, 7), (1024, 512, 11), (1536, 512, 15)]

BF = mybir.dt.bfloat16
F32 = mybir.dt.float32
EXP = mybir.ActivationFunctionType.Exp
MULT = mybir.AluOpType.mult
DIV = mybir.AluOpType.divide

_CACHE = {}


class _Ctx:
    """Shared build state so emission helpers stay small."""
    pass


def _emit_k_proj(cx, p, qb):
    """K projection for pair p, query strip qb -> KT[p][:, qb cols]."""
    nc = cx.nc
    ps = cx.projps.tile([128, 512], F32, tag="projp", name="kps")
    for ec in range(N_EC):
        nc.tensor.matmul(
            ps[:], cx.wqk[ec][:, p * 256 + 128:p * 256 + 256],
            cx.xT[ec][:, qb * 512:(qb + 1) * 512],
            start=(ec == 0), stop=(ec == N_EC - 1))
    nc.vector.tensor_copy(cx.KT[p][:, qb * 512:(qb + 1) * 512], ps[:])


def _emit_q_proj(cx, p, qb):
    nc = cx.nc
    ps = cx.projps.tile([128, 512], F32, tag="projp", name="qps")
    for ec in range(N_EC):
        nc.tensor.matmul(
            ps[:], cx.wqk[ec][:, p * 256:p * 256 + 128],
            cx.xT[ec][:, qb * 512:(qb + 1) * 512],
            start=(ec == 0), stop=(ec == N_EC - 1))
    nc.vector.tensor_copy(cx.QT[p][:, qb * 512:(qb + 1) * 512], ps[:])


def _emit_v_proj(cx, sb):
    """V projection for token block sb (all heads) + ones block.

    V per head is [128, 2*DH]: cols 0-63 the values, cols 64-127 all
    ones, so the P@V matmul's otherwise-idle output columns replicate
    the softmax denominator onto partitions 64-127 — no partition
    broadcast needed for the normalize.
    """
    nc = cx.nc
    ps = cx.projps.tile([128, EI_LOC], F32, tag="projp", name="vps")
    for ec in range(N_EC):
        nc.tensor.matmul(
            ps[:], cx.xT[ec][:, sb * 128:(sb + 1) * 128],
            cx.wv[ec][:], start=(ec == 0), stop=(ec == N_EC - 1))
    # full-tile ones first, then the values copy over cols 0-63 of each
    # head block (a strided memset of just the ones region mis-fills).
    nc.vector.memset(cx.V[sb][:], 1.0)
    nc.vector.tensor_copy(cx.V[sb][:, :, 0:DH], ps[:])


def _emit_out_proj(cx, sb):
    """Output projection for token block sb + bias; DMA to y_part."""
    nc = cx.nc
    for eo in range(2):
        ys = cx.projps.tile([128, 512], F32, tag="projp", name="ys")
        for c in range(4):
            nc.tensor.matmul(
                ys[:], cx.CT[c][:, sb * 128:(sb + 1) * 128],
                cx.woT[c][:, eo * 512:(eo + 1) * 512],
                start=(c == 0), stop=(c == 3))
        yt = cx.youtp.tile([128, 512], BF, tag="yt", name="yt")
        nc.vector.tensor_add(yt[:], ys[:], cx.bob[:, eo * 512:(eo + 1) * 512])
        nc.sync.dma_start(
            cx.y_part[sb * 128:(sb + 1) * 128, eo * 512:(eo + 1) * 512],
            yt[:])


def _emit_rs(cx, i):
    nc = cx.nc
    r0, n, _ = RS_CHUNKS[i]
    nc.gpsimd.collective_compute(
        "ReduceScatter", mybir.AluOpType.add,
        replica_groups=[[0, 1], [2, 3], [4, 5], [6, 7]],
        ins=[cx.y_part[r0:r0 + n, :]],
        outs=[cx.y_chunks[i][:]])


def _emit_y_out(cx, i):
    r0, n, _ = RS_CHUNKS[i]
    h = n // 2
    for j in range(4):
        cx.nc.sync.dma_start(
            cx.y_out[r0 // 2 + j * h // 4:r0 // 2 + (j + 1) * h // 4, :],
            cx.y_chunks[i][j * h // 4:(j + 1) * h // 4, :])


def _attention_strip(cx, p, qs, fillers):
    """Attention for head pair p over query strip qs (512 queries).

    fillers: dict kb -> list of emission closures run just before that
    kb iteration (PE filler work: projections, out-projections, RS).
    """
    nc = cx.nc
    qcols = slice(qs * 512, (qs + 1) * 512)
    pv_a = cx.pvps.tile([128, 512], F32, tag="pv", name="pva")
    pv_b = cx.pvps.tile([128, 512], F32, tag="pv", name="pvb")

    def emit_scores(kb):
        ps = cx.scps.tile([128, 1024], F32, tag="sc", name="sc")
        klo = slice(kb * 128, kb * 128 + 64)
        khi = slice(kb * 128 + 64, kb * 128 + 128)
        # head A (partitions 0-63 of QT/KT), head B (64-127); quadrant
        # tile positions are inferred from the operand base partitions.
        nc.tensor.matmul(ps[0:64, 0:512], cx.KT[p][0:64, klo],
                         cx.QT[p][0:64, qcols])
        nc.tensor.matmul(ps[64:128, 0:512], cx.KT[p][0:64, khi],
                         cx.QT[p][0:64, qcols])
        nc.tensor.matmul(ps[0:64, 512:1024], cx.KT[p][64:128, klo],
                         cx.QT[p][64:128, qcols])
        nc.tensor.matmul(ps[64:128, 512:1024], cx.KT[p][64:128, khi],
                         cx.QT[p][64:128, qcols])
        return ps

    # scores run one kb ahead of exp/PV so the ACT engine never waits
    # on the current iteration's score matmuls (rate-matches PE/ACT).
    pts = {}
    for fn in fillers.get(0, ()):
        fn()
    ps_next = emit_scores(0)
    for kb in range(N_KB):
        ps_cur = ps_next
        if kb + 1 < N_KB:
            for fn in fillers.get(kb + 1, ()):
                fn()
            ps_next = emit_scores(kb + 1)
        pt = cx.ptp.tile([128, 1024], BF, tag="pt", name="pt")
        nc.scalar.activation(pt[:], ps_cur[:], EXP, scale=cx.inv_sqrt_dh)
        pts[kb] = pt
        if kb >= 1:
            pkb = kb - 1
            nc.tensor.matmul(pv_a[:], cx.V[pkb][:, 2 * p, :],
                             pts[pkb][:, 0:512],
                             start=(pkb == 0), stop=False)
            nc.tensor.matmul(pv_b[:], cx.V[pkb][:, 2 * p + 1, :],
                             pts[pkb][:, 512:1024],
                             start=(pkb == 0), stop=False)
    nc.tensor.matmul(pv_a[:], cx.V[N_KB - 1][:, 2 * p, :],
                     pts[N_KB - 1][:, 0:512], start=False, stop=True)
    nc.tensor.matmul(pv_b[:], cx.V[N_KB - 1][:, 2 * p + 1, :],
                     pts[N_KB - 1][:, 512:1024], start=False, stop=True)

    # Drain pv to SBUF staging with two fast DVE copies so the PSUM
    # accumulators free immediately (the gpsimd-dependent normalize
    # below may lag behind a ReduceScatter on that queue; nothing in
    # the PE pipeline waits on it).
    st = cx.stp.tile([DH + 1, 1024], BF, tag="st", name="st")
    nc.vector.tensor_copy(st[:, 0:512], pv_a[0:DH + 1, :])
    nc.vector.tensor_copy(st[:, 512:1024], pv_b[0:DH + 1, :])

    # normalize: row DH of st holds the denominators. The broadcast
    # rides gpsimd; the multiplies run on gpsimd too so the Vector
    # queue never waits behind a collective-held gpsimd queue.
    den = cx.smallp.tile([1, 1024], F32, tag="den", name="den")
    nc.vector.tensor_copy(den[:], st[DH:DH + 1, :])
    rec1 = cx.smallp.tile([1, 1024], F32, tag="rec1", name="rec1")
    nc.vector.reciprocal_approx_fast(rec1[:], den[:])
    rec = cx.smallp.tile([64, 1024], F32, tag="rec", name="rec")
    nc.gpsimd.partition_broadcast(rec[:], rec1[:])
    nc.gpsimd.tensor_tensor(
        cx.CT[p][0:64, qcols], st[0:DH, 0:512], rec[:, 0:512], MULT)
    nc.gpsimd.tensor_tensor(
        cx.CT[p][64:128, qcols], st[0:DH, 512:1024], rec[:, 512:1024],
        MULT)


def _build():
    nc = bacc.Bacc("TRN2", target_bir_lowering=False, debug=False,
                   num_devices=N_CORES)

    xT_in = nc.declare_dram_parameter("xT", [E, S], BF, isOutput=False)
    wqk_in = nc.declare_dram_parameter("wqk", [E, 1024], BF, isOutput=False)
    wv_in = nc.declare_dram_parameter("wv", [E, EI_LOC], BF, isOutput=False)
    woT_in = nc.declare_dram_parameter("woT", [EI_LOC, E], BF, isOutput=False)
    bob_in = nc.declare_dram_parameter("bob", [128, E], F32, isOutput=False)
    y_out = nc.declare_dram_parameter("y", [S // TP, E], BF, isOutput=True)

    y_part = nc.dram_tensor("y_part", [S, E], BF)
    y_chunks = [nc.dram_tensor(f"y_chunk{i}", [n // 2, E], BF)
                for i, (_, n, _) in enumerate(RS_CHUNKS)]

    cx = _Ctx()
    cx.nc = nc
    cx.y_part, cx.y_chunks, cx.y_out = y_part, y_chunks, y_out
    cx.inv_sqrt_dh = 1.0 / float(np.sqrt(DH))

    with tile.TileContext(nc) as tc:
        with (
            tc.tile_pool(name="const", bufs=1) as constp,
            tc.tile_pool(name="persist", bufs=1) as persist,
            tc.tile_pool(name="projps", bufs=2, space="PSUM") as projps,
            tc.tile_pool(name="scps", bufs=2, space="PSUM") as scps,
            tc.tile_pool(name="pvps", bufs=2, space="PSUM") as pvps,
            tc.tile_pool(name="ptp", bufs=4) as ptp,
            tc.tile_pool(name="smallp", bufs=3) as smallp,
            tc.tile_pool(name="stp", bufs=3) as stp,
            tc.tile_pool(name="youtp", bufs=4) as youtp,
        ):
            cx.projps, cx.scps, cx.pvps = projps, scps, pvps
            cx.ptp, cx.smallp, cx.youtp = ptp, smallp, youtp
            cx.stp = stp

            # ---- input DMAs, ordered by first use ----
            # xT column halves: 2KB rows; first half feeds qb0/qb1.
            cx.xT = [constp.tile([128, S], BF, tag=f"xT{ec}",
                                 name=f"xT{ec}") for ec in range(N_EC)]
            for ec in range(N_EC):
                nc.sync.dma_start(cx.xT[ec][:, 0:512],
                                  xT_in[ec * 128:(ec + 1) * 128, 0:512])
            # pair-0 q/k weights (cols 0:256 of wqk)
            cx.wqk = [constp.tile([128, 1024], BF, tag=f"wqk{ec}",
                                  name=f"wqk{ec}") for ec in range(N_EC)]
            for ec in range(N_EC):
                nc.sync.dma_start(cx.wqk[ec][:, 0:256],
                                  wqk_in[ec * 128:(ec + 1) * 128, 0:256])
            for ec in range(N_EC):
                nc.sync.dma_start(cx.xT[ec][:, 512:1024],
                                  xT_in[ec * 128:(ec + 1) * 128, 512:1024])
            for ec in range(N_EC):
                nc.sync.dma_start(cx.xT[ec][:, 1024:2048],
                                  xT_in[ec * 128:(ec + 1) * 128, 1024:2048])
            cx.wv = []
            for ec in range(N_EC):
                t = constp.tile([128, EI_LOC], BF, tag=f"wv{ec}",
                                name=f"wv{ec}")
                nc.sync.dma_start(t[:], wv_in[ec * 128:(ec + 1) * 128, :])
                cx.wv.append(t)
            for ec in range(N_EC):
                nc.sync.dma_start(cx.wqk[ec][:, 256:1024],
                                  wqk_in[ec * 128:(ec + 1) * 128, 256:1024])
            cx.woT = []
            for c in range(4):
                t = constp.tile([128, E], BF, tag=f"woT{c}", name=f"woT{c}")
                nc.sync.dma_start(t[:], woT_in[c * 128:(c + 1) * 128, :])
                cx.woT.append(t)
            cx.bob = constp.tile([128, E], F32, tag="bob")
            nc.sync.dma_start(cx.bob[:], bob_in[:])

            # persistent SBUF tiles
            cx.QT = [persist.tile([128, S], BF, tag=f"QT{p}", name=f"QT{p}")
                     for p in range(N_HP)]
            cx.KT = [persist.tile([128, S], BF, tag=f"KT{p}", name=f"KT{p}")
                     for p in range(N_HP)]
            cx.V = [persist.tile([128, H_LOC, 2 * DH], BF, tag=f"V{s}",
                                 name=f"V{s}") for s in range(N_SB)]
            cx.CT = [persist.tile([128, S], BF, tag=f"CT{c}", name=f"CT{c}")
                     for c in range(N_HP)]

            # ---- bootstrap projections: pair-0 K/Q for strip 0, V0/V1 ----
            _emit_k_proj(cx, 0, 0)
            _emit_q_proj(cx, 0, 0)
            _emit_v_proj(cx, 0)
            _emit_v_proj(cx, 1)

            # ---- filler schedules ----
            def F(*fns):
                return list(fns)

            def mk(fn, *args):
                return lambda: fn(cx, *args)

            # (pair, strip) -> {kb: [closures]}
            sched = {(p, qs): {} for p in range(N_HP) for qs in range(N_QB)}

            # pair0/strip0: remaining V blocks (k+2 ahead of use) and
            # K/Q strips needed soon.
            s00 = sched[(0, 0)]
            s00[0] = F(mk(_emit_k_proj, 0, 1))
            s00[1] = F(mk(_emit_q_proj, 0, 1))
            for kb in range(14):
                s00.setdefault(kb + 2, []).append(mk(_emit_v_proj, kb + 2))
            s00.setdefault(5, []).insert(0, mk(_emit_k_proj, 0, 2))
            s00.setdefault(9, []).insert(0, mk(_emit_k_proj, 0, 3))
            # pair0/strip1: pair-1 projections + pair-0 q strips 2,3
            s01 = sched[(0, 1)]
            s01[0] = F(mk(_emit_q_proj, 0, 2))
            s01[2] = F(mk(_emit_q_proj, 0, 3))
            s01[4] = F(mk(_emit_k_proj, 1, 0))
            s01[6] = F(mk(_emit_q_proj, 1, 0))
            s01[8] = F(mk(_emit_k_proj, 1, 1))
            s01[10] = F(mk(_emit_q_proj, 1, 1))
            s01[12] = F(mk(_emit_k_proj, 1, 2))
            s01[14] = F(mk(_emit_k_proj, 1, 3))
            # pair1 strips 0,1: pair-1 q strips + pair-2 projections
            s10 = sched[(1, 0)]
            s10[0] = F(mk(_emit_q_proj, 1, 2))
            s10[4] = F(mk(_emit_q_proj, 1, 3))
            s10[8] = F(mk(_emit_k_proj, 2, 0))
            s10[12] = F(mk(_emit_q_proj, 2, 0))
            s11 = sched[(1, 1)]
            s11[0] = F(mk(_emit_k_proj, 2, 1))
            s11[4] = F(mk(_emit_q_proj, 2, 1))
            s11[8] = F(mk(_emit_k_proj, 2, 2))
            s11[12] = F(mk(_emit_q_proj, 2, 2))
            # pair2 strips 0,1: rest of pair-2 + pair-3 projections
            sched[(0, 3)].setdefault(0, []).append(mk(_emit_y_out, 0))
            sched[(2, 3)].setdefault(0, []).append(mk(_emit_y_out, 1))
            s20 = sched[(2, 0)]
            s20[0] = F(mk(_emit_k_proj, 2, 3))
            s20[4] = F(mk(_emit_q_proj, 2, 3))
            s20[8] = F(mk(_emit_k_proj, 3, 0))
            s20[12] = F(mk(_emit_q_proj, 3, 0))
            s21 = sched[(2, 1)]
            s21[0] = F(mk(_emit_k_proj, 3, 1))
            s21[4] = F(mk(_emit_q_proj, 3, 1))
            s21[8] = F(mk(_emit_k_proj, 3, 2))
            s21[12] = F(mk(_emit_q_proj, 3, 2))
            # pair3 strip0: rest of pair-3
            s30 = sched[(3, 0)]
            s30[0] = F(mk(_emit_k_proj, 3, 3))
            s30[4] = F(mk(_emit_q_proj, 3, 3))
            # pair3 strip1 (end of qp0): out-projection strip0 + RS 0
            s31 = sched[(3, 1)]
            s31[2] = F(mk(_emit_out_proj, 0))
            s31[5] = F(mk(_emit_out_proj, 1))
            s31[8] = F(mk(_emit_out_proj, 2))
            s31[11] = F(mk(_emit_out_proj, 3))

            # qp1 runs strip-major; strip-1 out-projection + RS 1 in the
            # first strips, strip-2's in the strip-3 row.
            s02 = sched[(0, 2)]
            s02[0] = F(mk(_emit_rs, 0))
            s02[2] = F(mk(_emit_out_proj, 4))
            s02[6] = F(mk(_emit_out_proj, 5))
            s02[10] = F(mk(_emit_out_proj, 6))
            s02[14] = F(mk(_emit_out_proj, 7))
            sched[(2, 2)].setdefault(0, []).insert(0, mk(_emit_rs, 1))
            s03 = sched[(0, 3)]
            s03[2] = F(mk(_emit_out_proj, 8))
            s03[6] = F(mk(_emit_out_proj, 9))
            s03[10] = F(mk(_emit_out_proj, 10))
            s03[14] = F(mk(_emit_out_proj, 11))
            s13 = sched[(1, 3)]
            s13[0] = F(mk(_emit_rs, 2))

            # ---- main loops ----
            # qp0: pair-major (pipelines projections pair to pair)
            for p in range(N_HP):
                for qs in range(2):
                    _attention_strip(cx, p, qs, sched[(p, qs)])
            # qp1: strip-major (each 512-row output block completes as
            # early as possible so its ReduceScatter overlaps compute)
            for qs in range(2, 4):
                for p in range(N_HP):
                    _attention_strip(cx, p, qs, sched[(p, qs)])

            # tail: strip-3 out-projection + final two chunks. All y_out
            # DMAs sit at the back of the Sync queue so a pending RS
            # never blocks compute-feeding DMAs.
            _emit_y_out(cx, 2)
            _emit_out_proj(cx, 12)
            _emit_out_proj(cx, 13)
            _emit_out_proj(cx, 14)
            _emit_out_proj(cx, 15)
            _emit_rs(cx, 3)
            _emit_y_out(cx, 3)

    nc.finalize()
    return nc


def _get_nc():
    if "nc" not in _CACHE:
        _CACHE["nc"] = _build()
    return _CACHE["nc"]


def _make_in_maps(x, wq, wk, wv, wo, bo):
    bf16 = ml_dtypes.bfloat16
    x, wq, wk, wv, wo, bo = (np.asarray(a) for a in (x, wq, wk, wv, wo, bo))
    in_maps = []
    for c in range(N_CORES):
        b, g = c // TP, c % TP
        h0 = g * H_LOC
        xT_l = np.ascontiguousarray(x[b].T).astype(bf16)
        # wqk: per pair p, cols [p*256, p*256+128) = wq heads (2p,2p+1),
        # cols [p*256+128, p*256+256) = wk heads (2p,2p+1)
        wq_l = wq[h0:h0 + H_LOC].transpose(1, 0, 2).reshape(E, H_LOC, DH)
        wk_l = wk[h0:h0 + H_LOC].transpose(1, 0, 2).reshape(E, H_LOC, DH)
        wqk = np.empty((E, 1024), np.float32)
        for p in range(N_HP):
            wqk[:, p * 256:p * 256 + 128] = \
                wq_l[:, 2 * p:2 * p + 2].reshape(E, 128)
            wqk[:, p * 256 + 128:p * 256 + 256] = \
                wk_l[:, 2 * p:2 * p + 2].reshape(E, 128)
        wv_l = np.ascontiguousarray(
            wv[h0:h0 + H_LOC].transpose(1, 0, 2).reshape(E, EI_LOC))
        woT_l = np.ascontiguousarray(
            wo[:, g * EI_LOC:(g + 1) * EI_LOC].T)
        bob = np.broadcast_to(bo.astype(np.float32) / TP, (128, E)).copy()
        in_maps.append({
            "xT": xT_l, "wqk": wqk.astype(bf16), "wv": wv_l.astype(bf16),
            "woT": woT_l.astype(bf16), "bob": bob,
        })
    return in_maps


def _assemble(results):
    out = np.empty((B, S, E), dtype=np.float32)
    for c in range(N_CORES):
        b, g = c // TP, c % TP
        y = results[c]["y"]
        for r0, n, _ in RS_CHUNKS:
            half = n // 2
            out[b, r0 + g * half:r0 + (g + 1) * half, :] = \
                y[r0 // 2:r0 // 2 + half, :]
    return out


def kernel(x, wq, wk, wv, wo, bo):
    nc = _get_nc()
    in_maps = _make_in_maps(x, wq, wk, wv, wo, bo)
    res = run_bass_kernel_spmd(nc, in_maps, list(range(N_CORES)))
    return _assemble(res.results)


# revision 27
# speedup vs baseline: 1.1005x; 1.1005x over previous
"""Multi-head attention Trainium2 kernel (B=4, S=2048, E=1024, H=16).

Sharding: 8 cores = 4 batch groups x 2-way head tensor-parallel.
Core c handles batch b=c//2 and heads [g*8, g*8+8) with g=c%2.
Partial output projections are pair-summed with a chunked ReduceScatter
(5 chunks, fired as soon as each 512/256-row block's projection lands),
so the exposed collective tail is one 256-row chunk.

v2 layout highlights:
- Scores run as 64x64 quadrant-packed matmuls: a head PAIR shares one
  512-col moving stream (head A's d on partitions 0-63, head B's on
  64-127), roughly halving scores PE time vs the padded-K=128 form.
- QT/KT tiles hold a head pair per 128 partitions (d of head A on
  0-63, head B on 64-127) which is exactly the projection PSUM layout,
  so each projection block needs a single [128,512] copy.
- Q/K/V projections and the output projection are emitted as PE filler
  inside the attention kb-loops, so the ACT engine (exp, the
  second-busiest engine) starts ~20us into the kernel instead of
  after a serial projection phase.
- exp runs once per (pair, 512-q strip, key block) on a [128,1024]
  PSUM tile (head A cols 0-511, head B 512-1023); V carries a ones
  column so P@V also emits the softmax denominator row.
- All matmuls in bf16 (fp32 PSUM accumulate).
"""

import os
import sys

import numpy as np

for _p in ("/opt/trn_rl_repo", "/root/.axon_site/_ro/trn_rl_repo"):
    if os.path.isdir(_p) and _p not in sys.path:
        sys.path.append(_p)

import ml_dtypes  # noqa: E402
from concourse import bacc, mybir, tile  # noqa: E402
from concourse.bass_utils import run_bass_kernel_spmd  # noqa: E402

B, S, E, H, DH = 4, 2048, 1024, 16, 64
N_CORES = 8
TP = 2  # head-parallel factor within a batch
H_LOC = H // TP  # 8 heads per core
EI_LOC = H_LOC * DH  # 512 local rows of the concat dim
N_SB = S // 128  # 16 token blocks
N_EC = E // 128  # 8 contraction chunks
N_QB = S // 512  # 4 query strips of 512
N_KB = S // 128  # 16 key blocks
N_HP = H_LOC // 2  # 4 head pairs
# ReduceScatter chunks: (row0, nrows, fire_after_sb)
RS_CHUNKS = [(0, 512, 3), (512, 512, 7), (1024, 512, 11), (1536, 512, 15)]

BF = mybir.dt.bfloat16
F32 = mybir.dt.float32
EXP = mybir.ActivationFunctionType.Exp
MULT = mybir.AluOpType.mult
DIV = mybir.AluOpType.divide

_CACHE = {}


class _Ctx:
    """Shared build state so emission helpers stay small."""
    pass


def _emit_k_proj(cx, p, qb):
    """K projection for pair p, query strip qb -> KT[p][:, qb cols]."""
    nc = cx.nc
    ps = cx.projps.tile([128, 512], F32, tag="projp", name="kps")
    for ec in range(N_EC):
        nc.tensor.matmul(
            ps[:], cx.wqk[ec][:, p * 256 + 128:p * 256 + 256],
            cx.xT[ec][:, qb * 512:(qb + 1) * 512],
            start=(ec == 0), stop=(ec == N_EC - 1))
    nc.vector.tensor_copy(cx.KT[p][:, qb * 512:(qb + 1) * 512], ps[:])


def _emit_q_proj(cx, p, qb):
    nc = cx.nc
    ps = cx.projps.tile([128, 512], F32, tag="projp", name="qps")
    for ec in range(N_EC):
        nc.tensor.matmul(
            ps[:], cx.wqk[ec][:, p * 256:p * 256 + 128],
            cx.xT[ec][:, qb * 512:(qb + 1) * 512],
            start=(ec == 0), stop=(ec == N_EC - 1))
    nc.vector.tensor_copy(cx.QT[p][:, qb * 512:(qb + 1) * 512], ps[:])


def _emit_v_proj(cx, sb):
    """V projection for token block sb (all heads) + ones block.

    V per head is [128, 2*DH]: cols 0-63 the values, cols 64-127 all
    ones, so the P@V matmul's otherwise-idle output columns replicate
    the softmax denominator onto partitions 64-127 — no partition
    broadcast needed for the normalize.
    """
    nc = cx.nc
    ps = cx.projps.tile([128, EI_LOC], F32, tag="projp", name="vps")
    for ec in range(N_EC):
        nc.tensor.matmul(
            ps[:], cx.xT[ec][:, sb * 128:(sb + 1) * 128],
            cx.wv[ec][:], start=(ec == 0), stop=(ec == N_EC - 1))
    # full-tile ones first, then the values copy over cols 0-63 of each
    # head block (a strided memset of just the ones region mis-fills).
    nc.vector.memset(cx.V[sb][:], 1.0)
    nc.vector.tensor_copy(cx.V[sb][:, :, 0:DH], ps[:])


def _emit_out_proj(cx, sb):
    """Output projection for token block sb + bias; DMA to y_part."""
    nc = cx.nc
    for eo in range(2):
        ys = cx.projps.tile([128, 512], F32, tag="projp", name="ys")
        for c in range(4):
            nc.tensor.matmul(
                ys[:], cx.CT[c][:, sb * 128:(sb + 1) * 128],
                cx.woT[c][:, eo * 512:(eo + 1) * 512],
                start=(c == 0), stop=(c == 3))
        yt = cx.youtp.tile([128, 512], BF, tag="yt", name="yt")
        nc.vector.tensor_add(yt[:], ys[:], cx.bob[:, eo * 512:(eo + 1) * 512])
        nc.sync.dma_start(
            cx.y_part[sb * 128:(sb + 1) * 128, eo * 512:(eo + 1) * 512],
            yt[:])


def _emit_rs(cx, i):
    nc = cx.nc
    r0, n, _ = RS_CHUNKS[i]
    nc.gpsimd.collective_compute(
        "ReduceScatter", mybir.AluOpType.add,
        replica_groups=[[0, 1], [2, 3], [4, 5], [6, 7]],
        ins=[cx.y_part[r0:r0 + n, :]],
        outs=[cx.y_chunks[i][:]])


def _emit_y_out(cx, i):
    r0, n, _ = RS_CHUNKS[i]
    h = n // 2
    for j in range(4):
        cx.nc.sync.dma_start(
            cx.y_out[r0 // 2 + j * h // 4:r0 // 2 + (j + 1) * h // 4, :],
            cx.y_chunks[i][j * h // 4:(j + 1) * h // 4, :])


def _attention_strip(cx, p, qs, fillers):
    """Attention for head pair p over query strip qs (512 queries).

    fillers: dict kb -> list of emission closures run just before that
    kb iteration (PE filler work: projections, out-projections, RS).
    """
    nc = cx.nc
    qcols = slice(qs * 512, (qs + 1) * 512)
    pv_a = cx.pvps.tile([128, 512], F32, tag="pv", name="pva")
    pv_b = cx.pvps.tile([128, 512], F32, tag="pv", name="pvb")

    def emit_scores(kb):
        ps = cx.scps.tile([128, 1024], F32, tag="sc", name="sc")
        klo = slice(kb * 128, kb * 128 + 64)
        khi = slice(kb * 128 + 64, kb * 128 + 128)
        # head A (partitions 0-63 of QT/KT), head B (64-127); quadrant
        # tile positions are inferred from the operand base partitions.
        nc.tensor.matmul(ps[0:64, 0:512], cx.KT[p][0:64, klo],
                         cx.QT[p][0:64, qcols])
        nc.tensor.matmul(ps[64:128, 0:512], cx.KT[p][0:64, khi],
                         cx.QT[p][0:64, qcols])
        nc.tensor.matmul(ps[0:64, 512:1024], cx.KT[p][64:128, klo],
                         cx.QT[p][64:128, qcols])
        nc.tensor.matmul(ps[64:128, 512:1024], cx.KT[p][64:128, khi],
                         cx.QT[p][64:128, qcols])
        return ps

    # scores run one kb ahead of exp/PV so the ACT engine never waits
    # on the current iteration's score matmuls (rate-matches PE/ACT).
    pts = {}
    for fn in fillers.get(0, ()):
        fn()
    ps_next = emit_scores(0)
    for kb in range(N_KB):
        ps_cur = ps_next
        if kb + 1 < N_KB:
            for fn in fillers.get(kb + 1, ()):
                fn()
            ps_next = emit_scores(kb + 1)
        pt = cx.ptp.tile([128, 1024], BF, tag="pt", name="pt")
        nc.scalar.activation(pt[:], ps_cur[:], EXP, scale=cx.inv_sqrt_dh)
        pts[kb] = pt
        if kb >= 1:
            pkb = kb - 1
            nc.tensor.matmul(pv_a[:], cx.V[pkb][:, 2 * p, :],
                             pts[pkb][:, 0:512],
                             start=(pkb == 0), stop=False)
            nc.tensor.matmul(pv_b[:], cx.V[pkb][:, 2 * p + 1, :],
                             pts[pkb][:, 512:1024],
                             start=(pkb == 0), stop=False)
    nc.tensor.matmul(pv_a[:], cx.V[N_KB - 1][:, 2 * p, :],
                     pts[N_KB - 1][:, 0:512], start=False, stop=True)
    nc.tensor.matmul(pv_b[:], cx.V[N_KB - 1][:, 2 * p + 1, :],
                     pts[N_KB - 1][:, 512:1024], start=False, stop=True)

    # Drain pv to SBUF staging with two fast DVE copies so the PSUM
    # accumulators free immediately (the gpsimd-dependent normalize
    # below may lag behind a ReduceScatter on that queue; nothing in
    # the PE pipeline waits on it).
    st = cx.stp.tile([DH + 1, 1024], BF, tag="st", name="st")
    nc.vector.tensor_copy(st[:, 0:512], pv_a[0:DH + 1, :])
    nc.vector.tensor_copy(st[:, 512:1024], pv_b[0:DH + 1, :])

    # normalize: row DH of st holds the denominators. The broadcast
    # rides gpsimd; the multiplies run on gpsimd too so the Vector
    # queue never waits behind a collective-held gpsimd queue.
    den = cx.smallp.tile([1, 1024], F32, tag="den", name="den")
    nc.vector.tensor_copy(den[:], st[DH:DH + 1, :])
    denb = cx.smallp.tile([64, 1024], F32, tag="denb", name="denb")
    nc.gpsimd.partition_broadcast(denb[:], den[:])
    rec = cx.smallp.tile([64, 1024], F32, tag="rec", name="rec")
    nc.vector.reciprocal_approx_fast(rec[:], denb[:])
    nc.vector.tensor_tensor(
        cx.CT[p][0:64, qcols], st[0:DH, 0:512], rec[:, 0:512], MULT)
    nc.vector.tensor_tensor(
        cx.CT[p][64:128, qcols], st[0:DH, 512:1024], rec[:, 512:1024],
        MULT)


def _build():
    nc = bacc.Bacc("TRN2", target_bir_lowering=False, debug=False,
                   num_devices=N_CORES)

    xT_in = nc.declare_dram_parameter("xT", [E, S], BF, isOutput=False)
    wqk_in = nc.declare_dram_parameter("wqk", [E, 1024], BF, isOutput=False)
    wv_in = nc.declare_dram_parameter("wv", [E, EI_LOC], BF, isOutput=False)
    woT_in = nc.declare_dram_parameter("woT", [EI_LOC, E], BF, isOutput=False)
    bob_in = nc.declare_dram_parameter("bob", [128, E], F32, isOutput=False)
    y_out = nc.declare_dram_parameter("y", [S // TP, E], BF, isOutput=True)

    y_part = nc.dram_tensor("y_part", [S, E], BF)
    y_chunks = [nc.dram_tensor(f"y_chunk{i}", [n // 2, E], BF)
                for i, (_, n, _) in enumerate(RS_CHUNKS)]

    cx = _Ctx()
    cx.nc = nc
    cx.y_part, cx.y_chunks, cx.y_out = y_part, y_chunks, y_out
    cx.inv_sqrt_dh = 1.0 / float(np.sqrt(DH))

    with tile.TileContext(nc) as tc:
        with (
            tc.tile_pool(name="const", bufs=1) as constp,
            tc.tile_pool(name="persist", bufs=1) as persist,
            tc.tile_pool(name="projps", bufs=2, space="PSUM") as projps,
            tc.tile_pool(name="scps", bufs=2, space="PSUM") as scps,
            tc.tile_pool(name="pvps", bufs=2, space="PSUM") as pvps,
            tc.tile_pool(name="ptp", bufs=4) as ptp,
            tc.tile_pool(name="smallp", bufs=3) as smallp,
            tc.tile_pool(name="stp", bufs=3) as stp,
            tc.tile_pool(name="youtp", bufs=4) as youtp,
        ):
            cx.projps, cx.scps, cx.pvps = projps, scps, pvps
            cx.ptp, cx.smallp, cx.youtp = ptp, smallp, youtp
            cx.stp = stp

            # ---- input DMAs, ordered by first use ----
            # xT column halves: 2KB rows; first half feeds qb0/qb1.
            cx.xT = [constp.tile([128, S], BF, tag=f"xT{ec}",
                                 name=f"xT{ec}") for ec in range(N_EC)]
            for ec in range(N_EC):
                nc.sync.dma_start(cx.xT[ec][:, 0:512],
                                  xT_in[ec * 128:(ec + 1) * 128, 0:512])
            # pair-0 q/k weights (cols 0:256 of wqk)
            cx.wqk = [constp.tile([128, 1024], BF, tag=f"wqk{ec}",
                                  name=f"wqk{ec}") for ec in range(N_EC)]
            for ec in range(N_EC):
                nc.sync.dma_start(cx.wqk[ec][:, 0:256],
                                  wqk_in[ec * 128:(ec + 1) * 128, 0:256])
            for ec in range(N_EC):
                nc.sync.dma_start(cx.xT[ec][:, 512:1024],
                                  xT_in[ec * 128:(ec + 1) * 128, 512:1024])
            for ec in range(N_EC):
                nc.sync.dma_start(cx.xT[ec][:, 1024:2048],
                                  xT_in[ec * 128:(ec + 1) * 128, 1024:2048])
            cx.wv = []
            for ec in range(N_EC):
                t = constp.tile([128, EI_LOC], BF, tag=f"wv{ec}",
                                name=f"wv{ec}")
                nc.sync.dma_start(t[:], wv_in[ec * 128:(ec + 1) * 128, :])
                cx.wv.append(t)
            for ec in range(N_EC):
                nc.sync.dma_start(cx.wqk[ec][:, 256:1024],
                                  wqk_in[ec * 128:(ec + 1) * 128, 256:1024])
            cx.woT = []
            for c in range(4):
                t = constp.tile([128, E], BF, tag=f"woT{c}", name=f"woT{c}")
                nc.sync.dma_start(t[:], woT_in[c * 128:(c + 1) * 128, :])
                cx.woT.append(t)
            cx.bob = constp.tile([128, E], F32, tag="bob")
            nc.sync.dma_start(cx.bob[:], bob_in[:])
            cx.ones64 = constp.tile([1, 64], F32, tag="ones64")
            nc.vector.memset(cx.ones64[:], 1.0)

            # persistent SBUF tiles
            cx.QT = [persist.tile([128, S], BF, tag=f"QT{p}", name=f"QT{p}")
                     for p in range(N_HP)]
            cx.KT = [persist.tile([128, S], BF, tag=f"KT{p}", name=f"KT{p}")
                     for p in range(N_HP)]
            cx.V = [persist.tile([128, H_LOC, 2 * DH], BF, tag=f"V{s}",
                                 name=f"V{s}") for s in range(N_SB)]
            cx.CT = [persist.tile([128, S], BF, tag=f"CT{c}", name=f"CT{c}")
                     for c in range(N_HP)]

            # ---- bootstrap projections: pair-0 K/Q for strip 0, V0/V1 ----
            _emit_k_proj(cx, 0, 0)
            _emit_q_proj(cx, 0, 0)
            _emit_v_proj(cx, 0)
            _emit_v_proj(cx, 1)

            # ---- filler schedules ----
            def F(*fns):
                return list(fns)

            def mk(fn, *args):
                return lambda: fn(cx, *args)

            # (pair, strip) -> {kb: [closures]}
            sched = {(p, qs): {} for p in range(N_HP) for qs in range(N_QB)}

            # pair0/strip0: remaining V blocks (k+2 ahead of use) and
            # K/Q strips needed soon.
            s00 = sched[(0, 0)]
            s00[0] = F(mk(_emit_k_proj, 0, 1))
            s00[1] = F(mk(_emit_q_proj, 0, 1))
            for kb in range(14):
                s00.setdefault(kb + 2, []).append(mk(_emit_v_proj, kb + 2))
            s00.setdefault(5, []).insert(0, mk(_emit_k_proj, 0, 2))
            s00.setdefault(9, []).insert(0, mk(_emit_k_proj, 0, 3))
            # pair0/strip1: pair-1 projections + pair-0 q strips 2,3
            s01 = sched[(0, 1)]
            s01[0] = F(mk(_emit_q_proj, 0, 2))
            s01[2] = F(mk(_emit_q_proj, 0, 3))
            s01[4] = F(mk(_emit_k_proj, 1, 0))
            s01[6] = F(mk(_emit_q_proj, 1, 0))
            s01[8] = F(mk(_emit_k_proj, 1, 1))
            s01[10] = F(mk(_emit_q_proj, 1, 1))
            s01[12] = F(mk(_emit_k_proj, 1, 2))
            s01[14] = F(mk(_emit_k_proj, 1, 3))
            # pair1 strips 0,1: pair-1 q strips + pair-2 projections
            s10 = sched[(1, 0)]
            s10[0] = F(mk(_emit_q_proj, 1, 2))
            s10[4] = F(mk(_emit_q_proj, 1, 3))
            s10[8] = F(mk(_emit_k_proj, 2, 0))
            s10[12] = F(mk(_emit_q_proj, 2, 0))
            s11 = sched[(1, 1)]
            s11[0] = F(mk(_emit_k_proj, 2, 1))
            s11[4] = F(mk(_emit_q_proj, 2, 1))
            s11[8] = F(mk(_emit_k_proj, 2, 2))
            s11[12] = F(mk(_emit_q_proj, 2, 2))
            # pair2 strips 0,1: rest of pair-2 + pair-3 projections
            sched[(0, 3)].setdefault(0, []).append(mk(_emit_y_out, 0))
            sched[(2, 3)].setdefault(0, []).append(mk(_emit_y_out, 1))
            s20 = sched[(2, 0)]
            s20[0] = F(mk(_emit_k_proj, 2, 3))
            s20[4] = F(mk(_emit_q_proj, 2, 3))
            s20[8] = F(mk(_emit_k_proj, 3, 0))
            s20[12] = F(mk(_emit_q_proj, 3, 0))
            s21 = sched[(2, 1)]
            s21[0] = F(mk(_emit_k_proj, 3, 1))
            s21[4] = F(mk(_emit_q_proj, 3, 1))
            s21[8] = F(mk(_emit_k_proj, 3, 2))
            s21[12] = F(mk(_emit_q_proj, 3, 2))
            # pair3 strip0: rest of pair-3
            s30 = sched[(3, 0)]
            s30[0] = F(mk(_emit_k_proj, 3, 3))
            s30[4] = F(mk(_emit_q_proj, 3, 3))
            # pair3 strip1 (end of qp0): out-projection strip0 + RS 0
            s31 = sched[(3, 1)]
            s31[2] = F(mk(_emit_out_proj, 0))
            s31[5] = F(mk(_emit_out_proj, 1))
            s31[8] = F(mk(_emit_out_proj, 2))
            s31[11] = F(mk(_emit_out_proj, 3))

            # qp1 runs strip-major; strip-1 out-projection + RS 1 in the
            # first strips, strip-2's in the strip-3 row.
            s02 = sched[(0, 2)]
            s02[0] = F(mk(_emit_rs, 0))
            s02[2] = F(mk(_emit_out_proj, 4))
            s02[6] = F(mk(_emit_out_proj, 5))
            s02[10] = F(mk(_emit_out_proj, 6))
            s02[14] = F(mk(_emit_out_proj, 7))
            sched[(2, 2)].setdefault(0, []).insert(0, mk(_emit_rs, 1))
            s03 = sched[(0, 3)]
            s03[2] = F(mk(_emit_out_proj, 8))
            s03[6] = F(mk(_emit_out_proj, 9))
            s03[10] = F(mk(_emit_out_proj, 10))
            s03[14] = F(mk(_emit_out_proj, 11))
            s13 = sched[(1, 3)]
            s13[0] = F(mk(_emit_rs, 2))

            # ---- main loops ----
            # qp0: pair-major (pipelines projections pair to pair)
            for p in range(N_HP):
                for qs in range(2):
                    _attention_strip(cx, p, qs, sched[(p, qs)])
            # qp1: strip-major (each 512-row output block completes as
            # early as possible so its ReduceScatter overlaps compute)
            for qs in range(2, 4):
                for p in range(N_HP):
                    _attention_strip(cx, p, qs, sched[(p, qs)])

            # tail: strip-3 out-projection + final two chunks. All y_out
            # DMAs sit at the back of the Sync queue so a pending RS
            # never blocks compute-feeding DMAs.
            _emit_y_out(cx, 2)
            _emit_out_proj(cx, 12)
            _emit_out_proj(cx, 13)
            _emit_out_proj(cx, 14)
            _emit_out_proj(cx, 15)
            _emit_rs(cx, 3)
            _emit_y_out(cx, 3)

    nc.finalize()
    return nc


def _get_nc():
    if "nc" not in _CACHE:
        _CACHE["nc"] = _build()
    return _CACHE["nc"]


def _make_in_maps(x, wq, wk, wv, wo, bo):
    bf16 = ml_dtypes.bfloat16
    x, wq, wk, wv, wo, bo = (np.asarray(a) for a in (x, wq, wk, wv, wo, bo))
    in_maps = []
    for c in range(N_CORES):
        b, g = c // TP, c % TP
        h0 = g * H_LOC
        xT_l = np.ascontiguousarray(x[b].T).astype(bf16)
        # wqk: per pair p, cols [p*256, p*256+128) = wq heads (2p,2p+1),
        # cols [p*256+128, p*256+256) = wk heads (2p,2p+1)
        wq_l = wq[h0:h0 + H_LOC].transpose(1, 0, 2).reshape(E, H_LOC, DH)
        wk_l = wk[h0:h0 + H_LOC].transpose(1, 0, 2).reshape(E, H_LOC, DH)
        wqk = np.empty((E, 1024), np.float32)
        for p in range(N_HP):
            wqk[:, p * 256:p * 256 + 128] = \
                wq_l[:, 2 * p:2 * p + 2].reshape(E, 128)
            wqk[:, p * 256 + 128:p * 256 + 256] = \
                wk_l[:, 2 * p:2 * p + 2].reshape(E, 128)
        wv_l = np.ascontiguousarray(
            wv[h0:h0 + H_LOC].transpose(1, 0, 2).reshape(E, EI_LOC))
        woT_l = np.ascontiguousarray(
            wo[:, g * EI_LOC:(g + 1) * EI_LOC].T)
        bob = np.broadcast_to(bo.astype(np.float32) / TP, (128, E)).copy()
        in_maps.append({
            "xT": xT_l, "wqk": wqk.astype(bf16), "wv": wv_l.astype(bf16),
            "woT": woT_l.astype(bf16), "bob": bob,
        })
    return in_maps


def _assemble(results):
    out = np.empty((B, S, E), dtype=np.float32)
    for c in range(N_CORES):
        b, g = c // TP, c % TP
        y = results[c]["y"]
        for r0, n, _ in RS_CHUNKS:
            half = n // 2
            out[b, r0 + g * half:r0 + (g + 1) * half, :] = \
                y[r0 // 2:r0 // 2 + half, :]
    return out


def kernel(x, wq, wk, wv, wo, bo):
    nc = _get_nc()
    in_maps = _make_in_maps(x, wq, wk, wv, wo, bo)
    res = run_bass_kernel_spmd(nc, in_maps, list(range(N_CORES)))
    return _assemble(res.results)


# revision 29
# speedup vs baseline: 1.1726x; 1.0655x over previous
"""Multi-head attention Trainium2 kernel (B=4, S=2048, E=1024, H=16).

Sharding: 8 cores = 4 batch groups x 2-way head tensor-parallel.
Core c handles batch b=c//2 and heads [g*8, g*8+8) with g=c%2.
Partial output projections are pair-summed with a chunked ReduceScatter
(5 chunks, fired as soon as each 512/256-row block's projection lands),
so the exposed collective tail is one 256-row chunk.

v2 layout highlights:
- Scores run as 64x64 quadrant-packed matmuls: a head PAIR shares one
  512-col moving stream (head A's d on partitions 0-63, head B's on
  64-127), roughly halving scores PE time vs the padded-K=128 form.
- QT/KT tiles hold a head pair per 128 partitions (d of head A on
  0-63, head B on 64-127) which is exactly the projection PSUM layout,
  so each projection block needs a single [128,512] copy.
- Q/K/V projections and the output projection are emitted as PE filler
  inside the attention kb-loops, so the ACT engine (exp, the
  second-busiest engine) starts ~20us into the kernel instead of
  after a serial projection phase.
- exp runs once per (pair, 512-q strip, key block) on a [128,1024]
  PSUM tile (head A cols 0-511, head B 512-1023); V carries a ones
  column so P@V also emits the softmax denominator row.
- All matmuls in bf16 (fp32 PSUM accumulate).
"""

import os
import sys

import numpy as np

for _p in ("/opt/trn_rl_repo", "/root/.axon_site/_ro/trn_rl_repo"):
    if os.path.isdir(_p) and _p not in sys.path:
        sys.path.append(_p)

import ml_dtypes  # noqa: E402
from concourse import bacc, mybir, tile  # noqa: E402
from concourse.bass_utils import run_bass_kernel_spmd  # noqa: E402

B, S, E, H, DH = 4, 2048, 1024, 16, 64
N_CORES = 8
TP = 2  # head-parallel factor within a batch
H_LOC = H // TP  # 8 heads per core
EI_LOC = H_LOC * DH  # 512 local rows of the concat dim
N_SB = S // 128  # 16 token blocks
N_EC = E // 128  # 8 contraction chunks
N_QB = S // 512  # 4 query strips of 512
N_KB = S // 128  # 16 key blocks
N_HP = H_LOC // 2  # 4 head pairs
# ReduceScatter chunks: (row0, nrows, fire_after_sb)
RS_CHUNKS = [(0, 512, 3), (512, 512, 7), (1024, 512, 11), (1536, 512, 15)]

BF = mybir.dt.bfloat16
F32 = mybir.dt.float32
EXP = mybir.ActivationFunctionType.Exp
MULT = mybir.AluOpType.mult
DIV = mybir.AluOpType.divide

_CACHE = {}


class _Ctx:
    """Shared build state so emission helpers stay small."""
    pass


def _emit_k_proj(cx, p, qb):
    """K projection for pair p, query strip qb -> KT[p][:, qb cols]."""
    nc = cx.nc
    ps = cx.projps.tile([128, 512], F32, tag="projp", name="kps")
    for ec in range(N_EC):
        nc.tensor.matmul(
            ps[:], cx.wqk[ec][:, p * 256 + 128:p * 256 + 256],
            cx.xT[ec][:, qb * 512:(qb + 1) * 512],
            start=(ec == 0), stop=(ec == N_EC - 1))
    nc.vector.tensor_copy(cx.KT[p][:, qb * 512:(qb + 1) * 512], ps[:])


def _emit_q_proj(cx, p, qb):
    nc = cx.nc
    ps = cx.projps.tile([128, 512], F32, tag="projp", name="qps")
    for ec in range(N_EC):
        nc.tensor.matmul(
            ps[:], cx.wqk[ec][:, p * 256:p * 256 + 128],
            cx.xT[ec][:, qb * 512:(qb + 1) * 512],
            start=(ec == 0), stop=(ec == N_EC - 1))
    nc.vector.tensor_copy(cx.QT[p][:, qb * 512:(qb + 1) * 512], ps[:])


def _emit_v_proj(cx, sb):
    """V projection for token block sb (all heads) + ones block.

    V per head is [128, 2*DH]: cols 0-63 the values, cols 64-127 all
    ones, so the P@V matmul's otherwise-idle output columns replicate
    the softmax denominator onto partitions 64-127 — no partition
    broadcast needed for the normalize.
    """
    nc = cx.nc
    ps = cx.projps.tile([128, EI_LOC], F32, tag="projp", name="vps")
    for ec in range(N_EC):
        nc.tensor.matmul(
            ps[:], cx.xT[ec][:, sb * 128:(sb + 1) * 128],
            cx.wv[ec][:], start=(ec == 0), stop=(ec == N_EC - 1))
    # full-tile ones first, then the values copy over cols 0-63 of each
    # head block (a strided memset of just the ones region mis-fills).
    nc.vector.memset(cx.V[sb][:], 1.0)
    nc.vector.tensor_copy(cx.V[sb][:, :, 0:DH], ps[:])


def _emit_out_proj(cx, sb):
    """Output projection for token block sb + bias; DMA to y_part."""
    nc = cx.nc
    for eo in range(2):
        ys = cx.projps.tile([128, 512], F32, tag="projp", name="ys")
        for c in range(4):
            nc.tensor.matmul(
                ys[:], cx.CT[c][:, sb * 128:(sb + 1) * 128],
                cx.woT[c][:, eo * 512:(eo + 1) * 512],
                start=(c == 0), stop=(c == 3))
        yt = cx.youtp.tile([128, 512], BF, tag="yt", name="yt")
        nc.vector.tensor_add(yt[:], ys[:], cx.bob[:, eo * 512:(eo + 1) * 512])
        nc.sync.dma_start(
            cx.y_part[sb * 128:(sb + 1) * 128, eo * 512:(eo + 1) * 512],
            yt[:])


def _emit_rs(cx, i):
    nc = cx.nc
    r0, n, _ = RS_CHUNKS[i]
    nc.gpsimd.collective_compute(
        "ReduceScatter", mybir.AluOpType.add,
        replica_groups=[[0, 1], [2, 3], [4, 5], [6, 7]],
        ins=[cx.y_part[r0:r0 + n, :]],
        outs=[cx.y_chunks[i][:]])


def _emit_y_out(cx, i):
    r0, n, _ = RS_CHUNKS[i]
    h = n // 2
    for j in range(4):
        cx.nc.sync.dma_start(
            cx.y_out[r0 // 2 + j * h // 4:r0 // 2 + (j + 1) * h // 4, :],
            cx.y_chunks[i][j * h // 4:(j + 1) * h // 4, :])


def _attention_strip(cx, p, qs, fillers):
    """Attention for head pair p over query strip qs (512 queries).

    fillers: dict kb -> list of emission closures run just before that
    kb iteration (PE filler work: projections, out-projections, RS).
    """
    nc = cx.nc
    qcols = slice(qs * 512, (qs + 1) * 512)
    pv_a = cx.pvps.tile([128, 512], F32, tag="pv", name="pva")
    pv_b = cx.pvps.tile([128, 512], F32, tag="pv", name="pvb")

    def emit_scores(kb):
        ps = cx.scps.tile([128, 1024], F32, tag="sc", name="sc")
        klo = slice(kb * 128, kb * 128 + 64)
        khi = slice(kb * 128 + 64, kb * 128 + 128)
        # head A (partitions 0-63 of QT/KT), head B (64-127); quadrant
        # tile positions are inferred from the operand base partitions.
        nc.tensor.matmul(ps[0:64, 0:512], cx.KT[p][0:64, klo],
                         cx.QT[p][0:64, qcols])
        nc.tensor.matmul(ps[64:128, 0:512], cx.KT[p][0:64, khi],
                         cx.QT[p][0:64, qcols])
        nc.tensor.matmul(ps[0:64, 512:1024], cx.KT[p][64:128, klo],
                         cx.QT[p][64:128, qcols])
        nc.tensor.matmul(ps[64:128, 512:1024], cx.KT[p][64:128, khi],
                         cx.QT[p][64:128, qcols])
        return ps

    # scores run one kb ahead of exp/PV so the ACT engine never waits
    # on the current iteration's score matmuls (rate-matches PE/ACT).
    pts = {}
    for fn in fillers.get(0, ()):
        fn()
    ps_next = emit_scores(0)
    for kb in range(N_KB):
        ps_cur = ps_next
        if kb + 1 < N_KB:
            for fn in fillers.get(kb + 1, ()):
                fn()
            ps_next = emit_scores(kb + 1)
        pt = cx.ptp.tile([128, 1024], BF, tag="pt", name="pt")
        nc.scalar.activation(pt[:], ps_cur[:], EXP, scale=cx.inv_sqrt_dh)
        pts[kb] = pt
        if kb >= 1:
            pkb = kb - 1
            nc.tensor.matmul(pv_a[:], cx.V[pkb][:, 2 * p, :],
                             pts[pkb][:, 0:512],
                             start=(pkb == 0), stop=False)
            nc.tensor.matmul(pv_b[:], cx.V[pkb][:, 2 * p + 1, :],
                             pts[pkb][:, 512:1024],
                             start=(pkb == 0), stop=False)
    nc.tensor.matmul(pv_a[:], cx.V[N_KB - 1][:, 2 * p, :],
                     pts[N_KB - 1][:, 0:512], start=False, stop=True)
    nc.tensor.matmul(pv_b[:], cx.V[N_KB - 1][:, 2 * p + 1, :],
                     pts[N_KB - 1][:, 512:1024], start=False, stop=True)

    # Drain pv to SBUF staging with two fast DVE copies so the PSUM
    # accumulators free immediately (the gpsimd-dependent normalize
    # below may lag behind a ReduceScatter on that queue; nothing in
    # the PE pipeline waits on it).
    st = cx.stp.tile([DH + 1, 1024], BF, tag="st", name="st")
    nc.vector.tensor_copy(st[:, 0:512], pv_a[0:DH + 1, :])
    nc.vector.tensor_copy(st[:, 512:1024], pv_b[0:DH + 1, :])

    # normalize, phase 1: den row + gpsimd broadcast (the broadcast may
    # lag behind a collective holding the gpsimd queue).
    den = cx.smallp.tile([1, 1024], F32, tag="den", name="den")
    nc.vector.tensor_copy(den[:], st[DH:DH + 1, :])
    denb = cx.smallp.tile([64, 1024], F32, tag="denb", name="denb")
    nc.gpsimd.partition_broadcast(denb[:], den[:])

    # phase 2 (recip + multiplies) is deferred one strip: by the time
    # it lands on the Vector queue the broadcast has cleared even a
    # worst-case collective hold, so Vector never head-of-line blocks.
    def finish_normalize(p=p, qcols=qcols, st=st, denb=denb):
        rec = cx.smallp.tile([64, 1024], F32, tag="rec", name="rec")
        nc.vector.reciprocal_approx_fast(rec[:], denb[:])
        nc.vector.tensor_tensor(
            cx.CT[p][0:64, qcols], st[0:DH, 0:512], rec[:, 0:512], MULT)
        nc.vector.tensor_tensor(
            cx.CT[p][64:128, qcols], st[0:DH, 512:1024],
            rec[:, 512:1024], MULT)

    while cx.norm_q:
        cx.norm_q.pop(0)()
    cx.norm_q.append(finish_normalize)


def _build():
    nc = bacc.Bacc("TRN2", target_bir_lowering=False, debug=False,
                   num_devices=N_CORES)

    xT_in = nc.declare_dram_parameter("xT", [E, S], BF, isOutput=False)
    wqk_in = nc.declare_dram_parameter("wqk", [E, 1024], BF, isOutput=False)
    wv_in = nc.declare_dram_parameter("wv", [E, EI_LOC], BF, isOutput=False)
    woT_in = nc.declare_dram_parameter("woT", [EI_LOC, E], BF, isOutput=False)
    bob_in = nc.declare_dram_parameter("bob", [128, E], F32, isOutput=False)
    y_out = nc.declare_dram_parameter("y", [S // TP, E], BF, isOutput=True)

    y_part = nc.dram_tensor("y_part", [S, E], BF)
    y_chunks = [nc.dram_tensor(f"y_chunk{i}", [n // 2, E], BF)
                for i, (_, n, _) in enumerate(RS_CHUNKS)]

    cx = _Ctx()
    cx.nc = nc
    cx.y_part, cx.y_chunks, cx.y_out = y_part, y_chunks, y_out
    cx.inv_sqrt_dh = 1.0 / float(np.sqrt(DH))

    with tile.TileContext(nc) as tc:
        with (
            tc.tile_pool(name="const", bufs=1) as constp,
            tc.tile_pool(name="persist", bufs=1) as persist,
            tc.tile_pool(name="projps", bufs=2, space="PSUM") as projps,
            tc.tile_pool(name="scps", bufs=2, space="PSUM") as scps,
            tc.tile_pool(name="pvps", bufs=2, space="PSUM") as pvps,
            tc.tile_pool(name="ptp", bufs=4) as ptp,
            tc.tile_pool(name="smallp", bufs=3) as smallp,
            tc.tile_pool(name="stp", bufs=3) as stp,
            tc.tile_pool(name="youtp", bufs=4) as youtp,
        ):
            cx.norm_q = []
            cx.projps, cx.scps, cx.pvps = projps, scps, pvps
            cx.ptp, cx.smallp, cx.youtp = ptp, smallp, youtp
            cx.stp = stp

            # ---- input DMAs, ordered by first use ----
            # xT column halves: 2KB rows; first half feeds qb0/qb1.
            cx.xT = [constp.tile([128, S], BF, tag=f"xT{ec}",
                                 name=f"xT{ec}") for ec in range(N_EC)]
            for ec in range(N_EC):
                nc.sync.dma_start(cx.xT[ec][:, 0:512],
                                  xT_in[ec * 128:(ec + 1) * 128, 0:512])
            # pair-0 q/k weights (cols 0:256 of wqk)
            cx.wqk = [constp.tile([128, 1024], BF, tag=f"wqk{ec}",
                                  name=f"wqk{ec}") for ec in range(N_EC)]
            for ec in range(N_EC):
                nc.sync.dma_start(cx.wqk[ec][:, 0:256],
                                  wqk_in[ec * 128:(ec + 1) * 128, 0:256])
            for ec in range(N_EC):
                nc.sync.dma_start(cx.xT[ec][:, 512:1024],
                                  xT_in[ec * 128:(ec + 1) * 128, 512:1024])
            for ec in range(N_EC):
                nc.sync.dma_start(cx.xT[ec][:, 1024:2048],
                                  xT_in[ec * 128:(ec + 1) * 128, 1024:2048])
            cx.wv = []
            for ec in range(N_EC):
                t = constp.tile([128, EI_LOC], BF, tag=f"wv{ec}",
                                name=f"wv{ec}")
                nc.sync.dma_start(t[:], wv_in[ec * 128:(ec + 1) * 128, :])
                cx.wv.append(t)
            for ec in range(N_EC):
                nc.sync.dma_start(cx.wqk[ec][:, 256:1024],
                                  wqk_in[ec * 128:(ec + 1) * 128, 256:1024])
            cx.woT = []
            for c in range(4):
                t = constp.tile([128, E], BF, tag=f"woT{c}", name=f"woT{c}")
                nc.sync.dma_start(t[:], woT_in[c * 128:(c + 1) * 128, :])
                cx.woT.append(t)
            cx.bob = constp.tile([128, E], F32, tag="bob")
            nc.sync.dma_start(cx.bob[:], bob_in[:])
            cx.ones64 = constp.tile([1, 64], F32, tag="ones64")
            nc.vector.memset(cx.ones64[:], 1.0)

            # persistent SBUF tiles
            cx.QT = [persist.tile([128, S], BF, tag=f"QT{p}", name=f"QT{p}")
                     for p in range(N_HP)]
            cx.KT = [persist.tile([128, S], BF, tag=f"KT{p}", name=f"KT{p}")
                     for p in range(N_HP)]
            cx.V = [persist.tile([128, H_LOC, 2 * DH], BF, tag=f"V{s}",
                                 name=f"V{s}") for s in range(N_SB)]
            cx.CT = [persist.tile([128, S], BF, tag=f"CT{c}", name=f"CT{c}")
                     for c in range(N_HP)]

            # ---- bootstrap projections: pair-0 K/Q for strip 0, V0/V1 ----
            _emit_k_proj(cx, 0, 0)
            _emit_q_proj(cx, 0, 0)
            _emit_v_proj(cx, 0)
            _emit_v_proj(cx, 1)

            # ---- filler schedules ----
            def F(*fns):
                return list(fns)

            def mk(fn, *args):
                return lambda: fn(cx, *args)

            # (pair, strip) -> {kb: [closures]}
            sched = {(p, qs): {} for p in range(N_HP) for qs in range(N_QB)}

            # pair0/strip0: remaining V blocks (k+2 ahead of use) and
            # K/Q strips needed soon.
            s00 = sched[(0, 0)]
            s00[0] = F(mk(_emit_k_proj, 0, 1))
            s00[1] = F(mk(_emit_q_proj, 0, 1))
            for kb in range(14):
                s00.setdefault(kb + 2, []).append(mk(_emit_v_proj, kb + 2))
            s00.setdefault(5, []).insert(0, mk(_emit_k_proj, 0, 2))
            s00.setdefault(9, []).insert(0, mk(_emit_k_proj, 0, 3))
            # pair0/strip1: pair-1 projections + pair-0 q strips 2,3
            s01 = sched[(0, 1)]
            s01[0] = F(mk(_emit_q_proj, 0, 2))
            s01[2] = F(mk(_emit_q_proj, 0, 3))
            s01[4] = F(mk(_emit_k_proj, 1, 0))
            s01[6] = F(mk(_emit_q_proj, 1, 0))
            s01[8] = F(mk(_emit_k_proj, 1, 1))
            s01[10] = F(mk(_emit_q_proj, 1, 1))
            s01[12] = F(mk(_emit_k_proj, 1, 2))
            s01[14] = F(mk(_emit_k_proj, 1, 3))
            # pair1 strips 0,1: pair-1 q strips + pair-2 projections
            s10 = sched[(1, 0)]
            s10[0] = F(mk(_emit_q_proj, 1, 2))
            s10[4] = F(mk(_emit_q_proj, 1, 3))
            s10[8] = F(mk(_emit_k_proj, 2, 0))
            s10[12] = F(mk(_emit_q_proj, 2, 0))
            s11 = sched[(1, 1)]
            s11[0] = F(mk(_emit_k_proj, 2, 1))
            s11[4] = F(mk(_emit_q_proj, 2, 1))
            s11[8] = F(mk(_emit_k_proj, 2, 2))
            s11[12] = F(mk(_emit_q_proj, 2, 2))
            # pair2 strips 0,1: rest of pair-2 + pair-3 projections
            sched[(0, 3)].setdefault(0, []).append(mk(_emit_y_out, 0))
            sched[(2, 3)].setdefault(0, []).append(mk(_emit_y_out, 1))
            s20 = sched[(2, 0)]
            s20[0] = F(mk(_emit_k_proj, 2, 3))
            s20[4] = F(mk(_emit_q_proj, 2, 3))
            s20[8] = F(mk(_emit_k_proj, 3, 0))
            s20[12] = F(mk(_emit_q_proj, 3, 0))
            s21 = sched[(2, 1)]
            s21[0] = F(mk(_emit_k_proj, 3, 1))
            s21[4] = F(mk(_emit_q_proj, 3, 1))
            s21[8] = F(mk(_emit_k_proj, 3, 2))
            s21[12] = F(mk(_emit_q_proj, 3, 2))
            # pair3 strip0: rest of pair-3
            s30 = sched[(3, 0)]
            s30[0] = F(mk(_emit_k_proj, 3, 3))
            s30[4] = F(mk(_emit_q_proj, 3, 3))
            # pair3 strip1 (end of qp0): out-projection strip0 + RS 0

            # qp1 runs strip-major; strip-1 out-projection + RS 1 in the
            # first strips, strip-2's in the strip-3 row.
            s02 = sched[(0, 2)]
            s02[2] = F(mk(_emit_out_proj, 0))
            s02[5] = F(mk(_emit_out_proj, 1))
            s02[8] = F(mk(_emit_out_proj, 2))
            s02[11] = F(mk(_emit_out_proj, 3))
            s12 = sched[(1, 2)]
            s12[0] = F(mk(_emit_rs, 0))
            s12[2] = F(mk(_emit_out_proj, 4))
            s12[6] = F(mk(_emit_out_proj, 5))
            s12[10] = F(mk(_emit_out_proj, 6))
            s12[14] = F(mk(_emit_out_proj, 7))
            sched[(2, 2)].setdefault(0, []).insert(0, mk(_emit_rs, 1))
            s13 = sched[(1, 3)]
            s13[2] = F(mk(_emit_out_proj, 8))
            s13[6] = F(mk(_emit_out_proj, 9))
            s13[10] = F(mk(_emit_out_proj, 10))
            s13[14] = F(mk(_emit_out_proj, 11))
            sched[(2, 3)].setdefault(0, []).insert(0, mk(_emit_rs, 2))

            # ---- main loops ----
            # qp0: pair-major (pipelines projections pair to pair)
            for p in range(N_HP):
                for qs in range(2):
                    _attention_strip(cx, p, qs, sched[(p, qs)])
            # qp1: strip-major (each 512-row output block completes as
            # early as possible so its ReduceScatter overlaps compute)
            for qs in range(2, 4):
                for p in range(N_HP):
                    _attention_strip(cx, p, qs, sched[(p, qs)])

            # tail: flush the deferred normalize, then strip-3
            # out-projection + final chunk.
            while cx.norm_q:
                cx.norm_q.pop(0)()
            _emit_y_out(cx, 2)
            _emit_out_proj(cx, 12)
            _emit_out_proj(cx, 13)
            _emit_out_proj(cx, 14)
            _emit_out_proj(cx, 15)
            _emit_rs(cx, 3)
            _emit_y_out(cx, 3)

    nc.finalize()
    return nc


def _get_nc():
    if "nc" not in _CACHE:
        _CACHE["nc"] = _build()
    return _CACHE["nc"]


def _make_in_maps(x, wq, wk, wv, wo, bo):
    bf16 = ml_dtypes.bfloat16
    x, wq, wk, wv, wo, bo = (np.asarray(a) for a in (x, wq, wk, wv, wo, bo))
    in_maps = []
    for c in range(N_CORES):
        b, g = c // TP, c % TP
        h0 = g * H_LOC
        xT_l = np.ascontiguousarray(x[b].T).astype(bf16)
        # wqk: per pair p, cols [p*256, p*256+128) = wq heads (2p,2p+1),
        # cols [p*256+128, p*256+256) = wk heads (2p,2p+1)
        wq_l = wq[h0:h0 + H_LOC].transpose(1, 0, 2).reshape(E, H_LOC, DH)
        wk_l = wk[h0:h0 + H_LOC].transpose(1, 0, 2).reshape(E, H_LOC, DH)
        wqk = np.empty((E, 1024), np.float32)
        for p in range(N_HP):
            wqk[:, p * 256:p * 256 + 128] = \
                wq_l[:, 2 * p:2 * p + 2].reshape(E, 128)
            wqk[:, p * 256 + 128:p * 256 + 256] = \
                wk_l[:, 2 * p:2 * p + 2].reshape(E, 128)
        wv_l = np.ascontiguousarray(
            wv[h0:h0 + H_LOC].transpose(1, 0, 2).reshape(E, EI_LOC))
        woT_l = np.ascontiguousarray(
            wo[:, g * EI_LOC:(g + 1) * EI_LOC].T)
        bob = np.broadcast_to(bo.astype(np.float32) / TP, (128, E)).copy()
        in_maps.append({
            "xT": xT_l, "wqk": wqk.astype(bf16), "wv": wv_l.astype(bf16),
            "woT": woT_l.astype(bf16), "bob": bob,
        })
    return in_maps


def _assemble(results):
    out = np.empty((B, S, E), dtype=np.float32)
    for c in range(N_CORES):
        b, g = c // TP, c % TP
        y = results[c]["y"]
        for r0, n, _ in RS_CHUNKS:
            half = n // 2
            out[b, r0 + g * half:r0 + (g + 1) * half, :] = \
                y[r0 // 2:r0 // 2 + half, :]
    return out


def kernel(x, wq, wk, wv, wo, bo):
    nc = _get_nc()
    in_maps = _make_in_maps(x, wq, wk, wv, wo, bo)
    res = run_bass_kernel_spmd(nc, in_maps, list(range(N_CORES)))
    return _assemble(res.results)
